# revision 35
# baseline (speedup 1.0000x reference)
"""ColBERT MaxSim late-interaction scoring on 8 Trainium2 NeuronCores.

scores[q, d] = sum_m max_n ( (Q*q_mask)[q,m,:] . (D*d_mask)[d,n,:] )

Sharding: candidate-parallel - the 512 docs are dealt across the 8 cores
(64 docs each); Q is replicated. Each core computes its [32, 64] score
block on device; the host concatenates.

Device algorithm (per core):
  - Masks are folded into layout: invalid doc tokens are never shipped,
    invalid qm rows are dropped and the survivors packed into B blocks of
    128 partitions; q attribution happens via a 0/1 indicator matmul.
  - Data is shipped in fp8e4m3; matmuls run in DoubleRow perf mode
    (both h-halves of the contraction live on 64 partitions, 2x PE rate).
  - "uv" units additionally fold pairs of doc tokens ON THE PE using
      max(a, b) = (a+b)/2 + |a-b|/2:
    the host packs u=(d1+d2)/2 and v=(d1-d2)/2 columns; the PE emits
    sim_u and sim_v, the ACT engine computes |sim_v| -> SBUF (fp8), and
    the PE accumulates it back onto sim_u's PSUM region (identity
    matmul, start=False, deferred one tile so the PE never stalls on
    ACT). The PSUM region then holds per-pair maxes, so the DVE sees
    only HALF the sim volume and each PSUM bank serves twice the sims.
  - Per-doc max over the remaining tokens: DVE reduce_max straight off
    PSUM (GPSIMD cannot touch PSUM or run TensorTensor through walrus,
    and ACT staging does not relieve DVE enough to pay for itself, so
    the planner routes every tile R0 and balances DVE vs ACT by
    choosing which units get the uv treatment).
  - scores = wseg^T @ maxv accumulated over the B blocks in PSUM.

Padding is exact: a padded zero column yields sim 0 == the reference's
masked-out contribution; an odd valid count pairs (t, 0) and
u + |v| = max(sim_t, 0), exactly the reference's max over {sim_t, 0}.
"""

import sys

sys.path.insert(0, "/opt/trn_rl_repo")

import math
from contextlib import ExitStack

import numpy as np
import ml_dtypes

import concourse.bass as bass
import concourse.mybir as mybir
from concourse.tile import TileContext
from concourse.tile_rust import add_dep_helper
from concourse.vector_clock import ScopedClock, VectorClock

N_CORES = 8
H = 128
Q_N, M_N = 32, 32
D_N, T_N = 512, 180
P_DOCS = D_N // N_CORES          # 64 docs per core
BANK = 512                       # fp32 slots per PSUM bank per partition

F32 = mybir.dt.float32
BF16 = mybir.dt.bfloat16
F8 = mybir.dt.float8e4
NP_F8 = ml_dtypes.float8_e4m3
NP_BF16 = ml_dtypes.bfloat16

_N_PROCS = 27
_patched = False

# calibrated TimelineSim cost constants (ns)
DVE_C, GP_C, ACT_C, PE_C = 1.042, 1.39, 0.833, 0.417
DVE_OV, GP_OV, ACT_OV = 180.0, 156.0, 404.0


def _install_tile_patch():
    """walrus rejects >2 sem waits on one CTRL: split the Tile tail drain
    into one SP drain per outstanding proc (SP executes them in order)."""
    global _patched
    if _patched:
        return
    _patched = True

    def _split_drain_and_barrier(self, tick_clock, wait_clock):
        nc = self.nc
        g = tick_clock.global_clock
        for p in range(_N_PROCS):
            t = g[p]
            if t > 0:
                d = nc.sync.drain()
                wait_clock.add_sem_waits(
                    d.ins,
                    ScopedClock(
                        {
                            None: VectorClock(
                                [t if i == p else 0 for i in range(_N_PROCS)]
                            )
                        }
                    ),
                )
        nc.sync.drain()
        nc.all_engine_barrier()
        assert self.sems is not None
        popped = nc._tile_sem_poison_stack.pop()
        assert popped is self._sem_poison
        nc.clear_and_free_semaphores(list(self.sems.allocated().values()))
        nc.all_engine_barrier()

    TileContext._drain_and_barrier = _split_drain_and_barrier


DMA_B = 0.3855          # ns per byte per partition (cost-model DMA rate)
SP_DMA = 565.0          # SP sequencer time per HWDGE issue


def _route_cost(kind, npad, w):
    """Per-(unit,block) resource costs for each reduce route.

    Hardware restrictions (walrus): GPSIMD/Pool can neither read PSUM
    nor run TensorTensor/free-axis reduces at all, so the reduce is a
    two-engine affair: every tile's PSUM exit is a DVE reduce_max (R0)
    or an ACT copy to SBUF bf16 (S*), after which DVE pairwise
    tensor_max folds run at 2x (bf16 perf mode) before the final
    reduce.  Returns route -> (dve, gp, act, pe, dmaq, sp).
    n = PSUM elems to exit = npad * w (post uv-fold width for uv)."""
    n = npad * w
    routes = {
        "R0": (DVE_C * n + DVE_OV, 0.0, 0.0, 0.0, 0.0, 0.0),
        "S1": (DVE_C * (n / 4 + n / 2) + 2 * DVE_OV, 0.0,
               ACT_C * n + ACT_OV, 0.0, 0.0, 0.0),
        "S2": (DVE_C * (n / 4 + n / 8 + n / 4) + 3 * DVE_OV, 0.0,
               ACT_C * n + ACT_OV, 0.0, 0.0, 0.0),
    }
    if kind == "uv":
        for r in routes:
            dv, gp, ac, pe, dq, sp = routes[r]
            routes[r] = (dv, gp, ac + ACT_C * n + ACT_OV,
                         pe + PE_C * n, dq, sp)
    return routes


def _plan_units(wpos, B):
    """Greedily walk the (sorted, per-position) widths building 2-bank
    units, choosing uv vs plain per unit and a reduce route per
    (unit, block) so DVE / GP / ACT stay balanced.

    Unit tuple: (kind, start, nd, w, wh, dpx) where dpx = docs per bank
    (of the PSUM layout: w-wide for plain, wh-wide for uv)."""
    units, routes = [], []
    # engine preloads: PE = base matmul cost (DoubleRow: 0.5 cyc/col) for
    # ALL sims + ramp/segment slack; DMA = input dg stream; SP = dg issues
    total_sims = B * sum((int(w) + 7) // 8 * 8 for w in wpos)
    load = {"dve": 0.0, "gp": 0.0, "act": 0.0,
            "pe": total_sims * PE_C * 0.5 + 1200.0,
            "dma": total_sims / B * 2.0 * DMA_B, "sp": 8 * SP_DMA}
    p = 0
    while p < P_DOCS:
        w = max(int(wpos[p]), 2)
        w = (w + 7) // 8 * 8
        wh = w // 2
        dpu = max(1, BANK // wh)
        nd_uv = min(2 * dpu, P_DOCS - p)
        dpb = max(1, BANK // w)
        nd_pl = min(2 * dpb, P_DOCS - p)

        best = None
        for kind, nd, dpx, wr in (("uv", nd_uv, dpu, wh), ("plain", nd_pl, dpb, w)):
            dpx = min(dpx, (nd + 1) // 2)     # don't pad doc slots
            costs = _route_cost(kind, 2 * dpx, wr)
            costs = {"R0": costs["R0"]}
            trial = dict(load)
            rts = []
            for _b in range(B):
                bestr = None
                for r, (dv, gp, ac, pe, dq, sp) in costs.items():
                    t2 = dict(trial)
                    t2["dve"] += dv
                    t2["gp"] += gp
                    t2["act"] += ac
                    t2["pe"] += pe
                    t2["dma"] += dq
                    t2["sp"] += sp
                    mk = max(t2.values())
                    if bestr is None or mk < bestr[0]:
                        bestr = (mk, r, t2)
                rts.append(bestr[1])
                trial = bestr[2]
            # prefer the type that advances more docs per unit of makespan
            score = (max(trial.values()) - max(load.values()) + 1.0) / nd
            if best is None or score < best[0]:
                best = (score, kind, nd, dpx, wr, rts, trial)
        _, kind, nd, dpx, wr, rts, trial = best
        units.append((kind, p, nd, w, wh, dpx))
        routes.extend(rts)
        load = trial
        p += nd
    return units, routes


def _host_prep(Q, D, q_mask, d_mask):
    """Layout work: compaction gathers, uv sum/diff packing, fp8 casts."""
    Q = np.asarray(Q, dtype=np.float32)
    D = np.asarray(D, dtype=np.float32)
    q_mask = np.asarray(q_mask)
    d_mask = np.asarray(d_mask)

    # ---- query side: pack valid qm rows into B blocks of 128
    qm_flat = q_mask.reshape(-1) != 0
    idx = np.flatnonzero(qm_flat)
    V = len(idx)
    B = max(1, math.ceil(V / 128))
    Qf = Q.reshape(Q_N * M_N, H)
    qt_full = np.zeros((H, B * 128), np.float32)
    if V:
        qt_full[:, :V] = Qf[idx].T
    # fp8 + DoubleRow interleave [64, 2, B*128] with h = i*64 + p
    qt = np.ascontiguousarray(
        qt_full.reshape(2, 64, B * 128).transpose(1, 0, 2)
    ).astype(NP_F8)
    wseg = np.zeros((H, B * Q_N), NP_BF16)
    for j, r in enumerate(idx):
        wseg[j % 128, (j // 128) * Q_N + (r // M_N)] = 1.0

    # ---- doc side: sort docs by valid-token count (desc), deal round-robin
    cnt = (d_mask != 0).sum(axis=1)
    order = np.argsort(-cnt, kind="stable")
    wpos = cnt[order[0::N_CORES]]                       # width at each position

    units, routes = _plan_units(wpos, B)

    # dg layout: per unit, uv: [u-bank0|u-bank1|v-bank0|v-bank1] each
    # dpx*wh wide; plain: [bank0|bank1] each dpx*w wide.
    ucols = []
    for kind, start, nd, w, wh, dpx in units:
        ucols.append(4 * dpx * wh if kind == "uv" else 2 * dpx * w)
    offs = np.cumsum([0] + ucols)
    total_cols = int(offs[-1])
    base_cols = np.cumsum([0] + [2 * dpx for _, _, _, _, _, dpx in units])
    p_pad = int(base_cols[-1])

    tok_idx = [np.flatnonzero(d_mask[d]) for d in range(D_N)]
    dgs = []
    for c in range(N_CORES):
        dg = np.zeros((H, total_cols), np.float32)
        for u, (kind, start, nd, w, wh, dpx) in enumerate(units):
            off = int(offs[u])
            for k in range(nd):
                doc = order[(start + k) * N_CORES + c]
                tk = tok_idx[doc]
                bank, slot = k // dpx, k % dpx
                if kind == "plain":
                    col = off + bank * dpx * w + slot * w
                    if len(tk):
                        dg[:, col : col + len(tk)] = D[doc][tk].T
                else:
                    toks = D[doc][tk].T                 # [H, nt]
                    nt = toks.shape[1]
                    npair = (nt + 1) // 2
                    uu = np.zeros((H, npair), np.float32)
                    vv = np.zeros((H, npair), np.float32)
                    uu[:, : nt // 2] = (toks[:, 0::2][:, : nt // 2] + toks[:, 1::2]) * 0.5
                    vv[:, : nt // 2] = (toks[:, 0::2][:, : nt // 2] - toks[:, 1::2]) * 0.5
                    if nt % 2:
                        uu[:, -1] = toks[:, -1] * 0.5
                        vv[:, -1] = toks[:, -1] * 0.5
                    ucol = off + bank * dpx * wh + slot * wh
                    vcol = off + 2 * dpx * wh + bank * dpx * wh + slot * wh
                    dg[:, ucol : ucol + npair] = uu
                    dg[:, vcol : vcol + npair] = vv
        dgs.append(
            np.ascontiguousarray(
                dg.reshape(2, 64, total_cols).transpose(1, 0, 2)
            ).astype(NP_F8)
        )

    ident = np.eye(H, dtype=NP_F8)

    return dict(
        qt=qt, wseg=wseg, dgs=dgs, ident=ident, units=units, routes=routes,
        offs=offs, base_cols=base_cols, total_cols=total_cols, p_pad=p_pad,
        B=B, order=order,
    )


def _build_program(B, units, routes, offs, base_cols, total_cols, p_pad,
                   repeats=1, compute_passes=1):
    """One SPMD program; per-core data comes via in_maps."""
    _install_tile_patch()
    nc = bass.Bass(trn_type="TRN2")
    qt_d = nc.dram_tensor("qt", [64, 2, B * 128], F8, kind="ExternalInput")
    wseg_d = nc.dram_tensor("wseg", [H, B * Q_N], BF16, kind="ExternalInput")
    dg_d = nc.dram_tensor("dg", [64, 2, total_cols], F8, kind="ExternalInput")
    id_d = nc.dram_tensor("ident", [H, H], F8, kind="ExternalInput")
    out_d = nc.dram_tensor("out", [Q_N, p_pad], F32, kind="ExternalOutput")

    DR = mybir.MatmulPerfMode.DoubleRow

    with TileContext(nc) as tc, ExitStack() as ctx:
        const = ctx.enter_context(tc.tile_pool(name="const", bufs=1))
        dpool = ctx.enter_context(tc.tile_pool(name="dg", bufs=1))
        apool = ctx.enter_context(tc.tile_pool(name="absv", bufs=2))
        fpool = ctx.enter_context(tc.tile_pool(name="fold", bufs=3))
        spool = ctx.enter_context(tc.tile_pool(name="stage", bufs=2))
        spool32 = ctx.enter_context(tc.tile_pool(name="stage32", bufs=2))
        mpool = ctx.enter_context(tc.tile_pool(name="maxv", bufs=2))
        opool = ctx.enter_context(tc.tile_pool(name="out", bufs=2))
        upool = ctx.enter_context(tc.tile_pool(name="psu", bufs=2, space="PSUM"))
        vpool = ctx.enter_context(tc.tile_pool(name="psv", bufs=2, space="PSUM"))

        # qt gates the first matmul: issue it before the dg stream; the
        # HWDGE descriptor generator is serial (~630ns per DMA), so order
        # matters. wseg/ident are needed only later and are issued after
        # the dg DMAs below.
        qt_t = const.tile([64, 2, B * 128], F8, tag="qt")
        qdma = nc.scalar.dma_start(out=qt_t[:], in_=qt_d[:, :, :])
        wseg_t = const.tile([H, B * Q_N], BF16, tag="wseg")
        id_t = const.tile([H, H], F8, tag="ident")
        n = nc.tensor.nop(hint="absorb_dma_wait")
        add_dep_helper(n.ins, qdma.ins, sync=True)

        def pe_guard(*insts):
            """Absorb cross-engine waits into PE nops so the matmul
            encoding (room for ~1 sem wait) never overflows."""
            for inst in insts:
                if inst is None:
                    continue
                n = nc.tensor.nop(hint="pe_guard")
                add_dep_helper(n.ins, inst.ins, sync=True)

        def act_guard(*insts):
            for inst in insts:
                if inst is None:
                    continue
                n = nc.scalar.nop(hint="act_guard")
                add_dep_helper(n.ins, inst.ins, sync=True)

        def gp_guard(*insts):
            for inst in insts:
                if inst is None:
                    continue
                n = nc.gpsimd.nop(hint="gp_guard")
                add_dep_helper(n.ins, inst.ins, sync=True)

        def ve_guard(*insts):
            for inst in insts:
                if inst is None:
                    continue
                n = nc.vector.nop(hint="ve_guard")
                add_dep_helper(n.ins, inst.ins, sync=True)

        hist = {"st": [], "f1": [], "f2": [], "av": []}

        def slot_guard(guard_fn, key, bufs):
            lst = hist[key]
            if len(lst) >= bufs and lst[-bufs] is not None:
                guard_fn(lst[-bufs])

        nu = len(units)
        u_hist = {}          # flat id -> (last psu writer, psu reader)
        v_hist = {}          # uv id -> (last psv writer, abs)
        av_hist = {}         # uv id -> the PE add reading that absv slot
        flat = 0             # global upool-ring counter (pso shares the ring)
        uvflat = 0
        const_dmas = {}      # lazily-absorbed const loads
        prev_out_copy = None
        prev_dg_readers = None
        for _rep in range(repeats):
            dg_tiles, dg_dmas = [], []
            for u, (kind, start, nd, w, wh, dpx) in enumerate(units):
                cols = 4 * dpx * wh if kind == "uv" else 2 * dpx * w
                dt = dpool.tile([64, 2, cols], F8, tag=f"dg{u}")
                if prev_dg_readers is not None and prev_dg_readers[u] is not None:
                    sn = nc.sync.nop(hint="sp_guard")
                    add_dep_helper(sn.ins, prev_dg_readers[u].ins, sync=True)
                udma = nc.sync.dma_start(
                    out=dt[:], in_=dg_d[:, :, int(offs[u]) : int(offs[u]) + cols]
                )
                dg_tiles.append(dt)
                dg_dmas.append(udma)
            if _rep == 0:
                wdma = nc.scalar.dma_start(out=wseg_t[:], in_=wseg_d[:, :])
                idma = nc.scalar.dma_start(out=id_t[:], in_=id_d[:, :])
                for dma in (wdma, idma):
                    n = nc.tensor.nop(hint="absorb_dma_wait")
                    add_dep_helper(n.ins, dma.ins, sync=True)
            dg_readers = [None] * nu

            for _pass in range(compute_passes):
                maxv = mpool.tile([H, B, p_pad], BF16, tag="maxv")
                reduces = []

                def emit_route(f, route, psu, last_mm, dpx, wr, b, base):
                    """PSUM-exit + fold + reduce for one tile."""
                    npad = 2 * dpx
                    h2, h4 = wr // 2, wr // 4
                    ps4 = (
                        psu[:, :]
                        .rearrange("p (nb bank) -> p nb bank", bank=BANK)[
                            :, :, 0 : dpx * wr
                        ]
                        .rearrange("p nb (d w) -> p nb d w", w=wr)
                    )
                    if route == "R0":      # DVE reduce straight off PSUM
                        rd = nc.vector.reduce_max(
                            out=maxv[:, b, base : base + npad].rearrange(
                                "p (nb d) -> p nb d", d=dpx
                            ),
                            in_=ps4,
                            axis=mybir.AxisListType.X,
                        )
                        u_hist[f] = (last_mm, rd)
                    elif route in ("S1", "S2"):  # ACT stage -> DVE folds
                        # TensorTensor allows only 2 free dims: flatten
                        # (bank, doc) -> one dim (uniform SBUF stride)
                        st = spool.tile([H, 2 * dpx * wr], BF16, tag="st")
                        slot_guard(act_guard, "st", 2)
                        act_guard(last_mm)
                        cp = nc.scalar.copy(
                            out=st[:].rearrange("p (nb x) -> p nb x", nb=2),
                            in_=psu[:, :].rearrange(
                                "p (nb bank) -> p nb bank", bank=BANK
                            )[:, :, 0 : dpx * wr],
                        )
                        u_hist[f] = (last_mm, cp)
                        stv = st[:].rearrange("p (g w) -> p g w", w=wr)
                        f1 = fpool.tile([H, 2 * dpx * h2], BF16, tag="f1")
                        f1v = f1[:].rearrange("p (g w) -> p g w", w=h2)
                        ve_guard(cp)
                        nc.vector.tensor_max(
                            f1v, stv[:, :, 0:h2], stv[:, :, h2:wr]
                        )
                        if route == "S1":
                            rd = nc.vector.reduce_max(
                                out=maxv[:, b, base : base + npad],
                                in_=f1v,
                                axis=mybir.AxisListType.X,
                            )
                        else:
                            f2 = fpool.tile([H, 2 * dpx * h4], BF16, tag="f2")
                            f2v = f2[:].rearrange("p (g w) -> p g w", w=h4)
                            nc.vector.tensor_max(
                                f2v, f1v[:, :, 0:h4], f1v[:, :, h4:h2]
                            )
                            rd = nc.vector.reduce_max(
                                out=maxv[:, b, base : base + npad],
                                in_=f2v,
                                axis=mybir.AxisListType.X,
                            )
                        hist["st"].append(rd)  # conservative: after the fold
                    else:
                        raise ValueError(f"unknown route {route}")
                    reduces.append(rd)

                def flush(pending):
                    """Deferred uv tail: PE add of |v| then the reduce route.
                    Deferring one tile keeps the PE busy on the next tile's
                    matmuls while ACT computes |v|."""
                    if pending is None:
                        return
                    (f, uf, route, psu, av, cp, dpx, wr, b, base, u) = pending
                    pe_guard(cp)
                    add = None
                    for k in range(2):
                        add = nc.tensor.matmul(
                            psu[:, k * BANK : k * BANK + dpx * wr],
                            lhsT=id_t[:, :],
                            rhs=av[:, k, :],
                            start=False,
                            stop=True,
                            skip_group_check=True,
                        )
                    av_hist[uf] = add
                    emit_route(f, route, psu, add, dpx, wr, b, base)

                pending = None
                for u, (kind, start, nd, w, wh, dpx) in enumerate(units):
                    base = int(base_cols[u])
                    wr = wh if kind == "uv" else w
                    dgt = dg_tiles[u]
                    for b in range(B):
                        f = flat
                        flat += 1
                        nc.tensor.nop(hint="spare")
                        nc.vector.nop(hint="spare")
                        if b == 0:
                            nc.sync.nop(hint="spare")
                        nc.scalar.nop(hint="spare")
                        guards = []
                        if f - 2 in u_hist:
                            guards.extend(u_hist[f - 2])
                        if b == 0:
                            guards.append(dg_dmas[u])
                        pe_guard(*guards)
                        psu = upool.tile([H, 2 * BANK], F32, tag="psu")
                        last_mm = None
                        # u (or plain) matmuls, one per bank
                        for k in range(2):
                            last_mm = nc.tensor.matmul(
                                psu[:, k * BANK : k * BANK + dpx * wr],
                                lhsT=qt_t[:, :, b * 128 : (b + 1) * 128],
                                rhs=dgt[:, :, k * dpx * wr : (k + 1) * dpx * wr],
                                start=True,
                                stop=(kind == "plain"),
                                perf_mode=DR,
                                skip_group_check=True,
                            )
                        route = routes[u * B + b]
                        if kind == "uv":
                            uf = uvflat
                            uvflat += 1
                            vg = []
                            if uf - 2 in v_hist:
                                vg.extend(v_hist[uf - 2])
                            pe_guard(*vg)
                            psv = vpool.tile([H, 2 * BANK], F32, tag="psv")
                            last_vmm = None
                            for k in range(2):
                                last_vmm = nc.tensor.matmul(
                                    psv[:, k * BANK : k * BANK + dpx * wr],
                                    lhsT=qt_t[:, :, b * 128 : (b + 1) * 128],
                                    rhs=dgt[:, :, 2 * dpx * wr + k * dpx * wr :
                                            3 * dpx * wr + k * dpx * wr],
                                    start=True,
                                    stop=True,
                                    perf_mode=DR,
                                    skip_group_check=True,
                                )
                            dg_readers[u] = last_vmm
                            # flush the previous tile BEFORE this tile's abs
                            # so its stage copy (if S-routed) isn't queued
                            # behind this abs on the ACT stream
                            flush(pending)
                            pending = None
                            av = apool.tile([H, 2, dpx * wr], F8, tag="absv")
                            if uf - 2 in av_hist:
                                act_guard(av_hist[uf - 2])
                            act_guard(last_vmm)
                            cp = nc.scalar.activation(
                                av[:],
                                psv[:, :].rearrange(
                                    "p (nb bank) -> p nb bank", bank=BANK
                                )[:, :, 0 : dpx * wr],
                                mybir.ActivationFunctionType.Abs,
                            )
                            v_hist[uf] = (last_vmm, cp)
                            pending = (f, uf, route, psu, av, cp, dpx, wr,
                                       b, base, u)
                        else:
                            dg_readers[u] = last_mm
                            flush(pending)
                            pending = None
                            emit_route(f, route, psu, last_mm, dpx, wr, b, base)
                flush(pending)
                pending = None

                pe_guard(prev_out_copy)
                seg_guard = nc.tensor.nop(hint="seg_guard")
                for r in reduces:
                    add_dep_helper(seg_guard.ins, r.ins, sync=True)
                fso = flat
                flat += 1
                if fso - 2 in u_hist:
                    pe_guard(*u_hist[fso - 2])
                pso = upool.tile([Q_N, p_pad], F32, tag="psu")
                seg_mm = None
                for b in range(B):
                    seg_mm = nc.tensor.matmul(
                        pso[:, :],
                        lhsT=wseg_t[:, b * Q_N : (b + 1) * Q_N],
                        rhs=maxv[:, b, :],
                        start=(b == 0),
                        stop=(b == B - 1),
                    )
                if _pass == compute_passes - 1:
                    out_t = opool.tile([Q_N, p_pad], F32, tag="out")
                    act_guard(seg_mm)
                    oc = nc.scalar.copy(out=out_t[:], in_=pso[:, :])
                    prev_out_copy = oc
                    u_hist[fso] = (seg_mm, oc)
                    sn = nc.sync.nop(hint="out_guard")
                    add_dep_helper(sn.ins, oc.ins, sync=True)
                    nc.sync.dma_start(out=out_d[:, :], in_=out_t[:])
                else:
                    u_hist[fso] = (seg_mm, seg_mm)
            prev_dg_readers = dg_readers

    _redistribute_waits(nc)
    return nc


# walrus encoding limits on sem waits per instruction
_WAIT_CAPS = {"InstMatmult": 1, "InstNoOp": 1, "InstDrain": 1,
              "InstDMACopy": 1, "InstTensorReduce": 1, "InstActivation": 1,
              "InstTensorTensor": 1, "InstMemset": 1, "InstTensorCopy": 1}


def _redistribute_waits(nc):
    """Move excess sem waits off over-limit instructions onto earlier
    instructions of the same engine (in final program order)."""
    import dataclasses

    fn = nc.m.functions[0]
    streams = {}
    for bb in fn.blocks:
        for inst in bb.instructions:
            eng = inst.engine
            streams.setdefault(str(eng), []).append(inst)

    for eng, insts in streams.items():
        for i, inst in enumerate(insts):
            cap = _WAIT_CAPS.get(type(inst).__name__)
            si = inst.sync_info
            if cap is None or si is None or len(si.on_wait) <= cap:
                continue
            eng_name = str(inst.engine).split(".")[-1]
            waits = sorted(
                si.on_wait,
                key=lambda w: 0 if w.ant_name.startswith(eng_name) else 1,
            )
            keep, excess = list(waits[:cap]), list(waits[cap:])
            if any(w.ant_name.startswith(eng_name) for w in excess):
                keep, excess = list(waits), []
            for w in excess:
                placed = False
                for j in range(i - 1, max(-1, i - 8), -1):
                    p = insts[j]
                    if type(p).__name__ not in (
                        "InstNoOp", "InstMatmult", "InstDrain",
                        "InstActivation", "InstTensorReduce", "InstTensorTensor",
                    ):
                        continue
                    pcap = _WAIT_CAPS.get(type(p).__name__, 1)
                    psi = p.sync_info
                    pw = list(psi.on_wait) if psi else []
                    merged = False
                    for k, ow in enumerate(pw):
                        if ow.id == w.id and ow.wait_mode == w.wait_mode == "sem-ge-imm":
                            pw[k] = dataclasses.replace(
                                ow, wait_value=max(ow.wait_value, w.wait_value)
                            )
                            merged = True
                            break
                    if not merged:
                        if len(pw) >= pcap:
                            continue
                        pw.append(w)
                    if psi is None:
                        psi = type(si)(on_wait=pw, on_update=[])
                    else:
                        psi = dataclasses.replace(psi, on_wait=pw)
                    p.sync_info = psi
                    placed = True
                    break
                if not placed:
                    keep.append(w)
            inst.sync_info = dataclasses.replace(si, on_wait=keep)


def _run(nc, prep, n_cores=N_CORES):
    from concourse.bass_utils import run_bass_kernel_spmd

    in_maps = [
        {"qt": prep["qt"], "wseg": prep["wseg"], "dg": prep["dgs"][c],
         "ident": prep["ident"]}
        for c in range(n_cores)
    ]
    res = run_bass_kernel_spmd(nc, in_maps, core_ids=list(range(n_cores)))
    return res.results


def _assemble(prep, results):
    order = prep["order"]
    base_cols = prep["base_cols"]
    scores = np.zeros((Q_N, D_N), np.float32)
    for c in range(N_CORES):
        out_c = results[c]["out"]
        for u, (kind, start, nd, w, wh, dpx) in enumerate(prep["units"]):
            base = int(base_cols[u])
            for k in range(nd):
                scores[:, order[(start + k) * N_CORES + c]] = out_c[:, base + k]
    return scores


_cache = {}


def kernel(Q, D, q_mask, d_mask):
    prep = _host_prep(Q, D, q_mask, d_mask)
    key = (prep["B"], tuple(prep["units"]), tuple(prep["routes"]))
    if key not in _cache:
        _cache[key] = _build_program(
            prep["B"], prep["units"], prep["routes"], prep["offs"],
            prep["base_cols"], prep["total_cols"], prep["p_pad"],
        )
    nc = _cache[key]
    results = _run(nc, prep)
    return _assemble(prep, results)
